# revision 9
# baseline (speedup 1.0000x reference)
"""Dependency-GCN message passing kernel for 8 Trainium2 NeuronCores.

Strategy (single SPMD program, no collectives):
  - Each core owns a contiguous range of 3750 destination nodes.
  - Host buckets every edge by (direction, owner-core), sorts by
    (dst-chunk, relation, dst), cuts 128-edge blocks limited to a
    3-window (384-node) destination frame, pads blocks with dummy
    edges, and equalizes block counts across cores so one program
    serves all 8 cores (all per-core variation lives in data).
  - Device: fp16 transpose-gather of source rows (dma_gather) ->
    per-block transform matmuls (G^T stationary, W streamed) ->
    PSUM -> fp16 copy -> one-hot selection matrix (is_equal vs iota)
    -> scatter matmuls accumulating into per-window PSUM slots
    addressed by a per-block register offset -> per-chunk flush into
    an SBUF accumulator initialized with all bias contributions ->
    DMA out.  Self-transform rides the pipeline as relation 20.
"""

import sys

if "/opt/trn_rl_repo" not in sys.path:
    sys.path.insert(0, "/opt/trn_rl_repo")

import numpy as np

import concourse.bacc as bacc
import concourse.mybir as mybir
from concourse.bass import ds
from concourse.tile import TileContext
from concourse.bass_utils import run_bass_kernel_spmd

F32 = mybir.dt.float32
F16 = mybir.dt.float16
I16 = mybir.dt.int16
I32 = mybir.dt.int32

N_NODES = 30000
N_REL = 10
D = 256
N_CORES = 8
NODES_PER_CORE = N_NODES // N_CORES          # 3750
WIN = 128                                     # nodes per PSUM window
N_WIN = (NODES_PER_CORE + WIN - 1) // WIN     # 30 (last partial)
NODES_PAD = N_WIN * WIN                       # 3840
CHUNK_WINS = [12, 12, N_WIN - 24]             # windows per chunk: 12,12,6
CHUNK_BASE_W = [0, 12, 24]
FRAME = 3                                     # windows per block frame
SELF_REL = 20                                 # W-slot for self transform
DUMMY_DSTREL = -1024.0                        # fp16-exact, never matches iota


# ---------------------------------------------------------------- host prep

def _pack_idx16(idx: np.ndarray) -> np.ndarray:
    """[n] -> [128, n//16] int16: idx i at (partition i%16, col i//16), x8."""
    n = idx.shape[0]
    t = idx.astype(np.int16).reshape(n // 16, 16).T
    return np.tile(t, (8, 1))


def _build_blocks(src, dst_local, chunk, nwin):
    """Cut dst-sorted edges of one (core, dir, chunk, rel) into blocks.

    Returns list of (src128, dstrel128, w0_local) with 128-entry arrays.
    """
    base = CHUNK_BASE_W[chunk] * WIN
    blocks = []
    i = 0
    n = src.shape[0]
    while i < n:
        w_first = int(dst_local[i] - base) // WIN
        w0 = min(w_first, nwin - FRAME)
        lo = base + w0 * WIN
        # edges fitting the 3-window frame, up to 128
        j = min(i + 128, n)
        # dst_local sorted: find first edge beyond frame
        hi = lo + FRAME * WIN
        j = min(j, i + int(np.searchsorted(dst_local[i:j], hi)))
        k = j - i
        s = np.zeros(128, np.int16)
        r = np.full(128, DUMMY_DSTREL, np.float16)
        s[:k] = src[i:j]
        r[:k] = (dst_local[i:j] - lo).astype(np.float16)
        blocks.append((s, r, w0))
        i = j
    return blocks


def _dummy_block():
    return (np.zeros(128, np.int16),
            np.full(128, DUMMY_DSTREL, np.float16), 0)


def prepare(x, W_self, b_self, W_fwd, b_fwd, W_rev, b_rev,
            dep_idx, rel_idx, gov_idx):
    """All host-side sharding/packing. Returns (schedule, per-core in_maps,
    shared arrays)."""
    dep_idx = np.asarray(dep_idx).astype(np.int64)
    rel_idx = np.asarray(rel_idx).astype(np.int64)
    gov_idx = np.asarray(gov_idx).astype(np.int64)
    x = np.asarray(x, np.float32)

    x16 = x.astype(np.float16)

    # weight stack [128, 21*2*256] fp16: col = (relW*2 + half)*256 + o
    W_all = np.zeros((21, D, D), np.float32)
    W_all[0:10] = np.asarray(W_fwd, np.float32)
    W_all[10:20] = np.asarray(W_rev, np.float32)
    W_all[20] = np.asarray(W_self, np.float32)
    wsb = np.zeros((128, 21 * 2 * D), np.float16)
    for rw in range(21):
        for h in range(2):
            wsb[:, (rw * 2 + h) * D:(rw * 2 + h + 1) * D] = \
                W_all[rw, h * 128:(h + 1) * 128, :].astype(np.float16)

    # iota for S generation: [128, 384] fp16 (same value down partitions)
    iota = np.tile(np.arange(FRAME * WIN, dtype=np.float16), (128, 1))

    # per-(core, dir) edge lists
    # fwd: src=gov, dst=dep, relW=rel ; rev: src=dep, dst=gov, relW=rel+10
    core_blocks = [[[] for _ in range(2)] for _ in range(N_CORES)]
    # [core][dir] -> dict[(chunk, relW)] -> list of blocks
    for d in range(2):
        if d == 0:
            src_a, dst_a, relw_a = gov_idx, dep_idx, rel_idx
        else:
            src_a, dst_a, relw_a = dep_idx, gov_idx, rel_idx + 10
        core_of = dst_a // NODES_PER_CORE
        for c in range(N_CORES):
            m = core_of == c
            src_c = src_a[m]
            dst_c = dst_a[m] - c * NODES_PER_CORE
            rel_c = relw_a[m]
            order = np.lexsort((dst_c, rel_c))
            src_c, dst_c, rel_c = src_c[order], dst_c[order], rel_c[order]
            chunk_c = np.minimum(dst_c // (12 * WIN), 2)
            d_blocks = {}
            for ch in range(3):
                nwin = CHUNK_WINS[ch]
                mc = chunk_c == ch
                sc, dc, rc = src_c[mc], dst_c[mc], rel_c[mc]
                for rw in np.unique(rc):
                    mr = rc == rw
                    d_blocks[(ch, int(rw))] = _build_blocks(
                        sc[mr], dc[mr], ch, nwin)
                # self blocks (fwd only): one per window
                if d == 0:
                    sb = []
                    for wl in range(nwin):
                        g0 = c * NODES_PER_CORE + (CHUNK_BASE_W[ch] + wl) * WIN
                        n_real = min(WIN, NODES_PER_CORE -
                                     (CHUNK_BASE_W[ch] + wl) * WIN)
                        if n_real <= 0:
                            continue
                        s = np.zeros(128, np.int16)
                        r = np.full(128, DUMMY_DSTREL, np.float16)
                        s[:n_real] = np.arange(g0, g0 + n_real, dtype=np.int16)
                        w0 = min(wl, nwin - FRAME)
                        r[:n_real] = ((wl - w0) * WIN +
                                      np.arange(n_real)).astype(np.float16)
                        sb.append((s, r, w0))
                    d_blocks[(ch, SELF_REL)] = sb
            core_blocks[c][d] = d_blocks

    # equalized schedule: ordered list of (dir, chunk, relW, nblocks)
    schedule = []
    for d in range(2):
        rel_order = ([SELF_REL] + list(range(10))) if d == 0 \
            else list(range(10, 20))
        for ch in range(3):
            for rw in rel_order:
                nb = max(len(core_blocks[c][d].get((ch, rw), []))
                         for c in range(N_CORES))
                if nb > 0:
                    schedule.append((d, ch, rw, nb))

    # assemble per-core flat arrays in schedule order
    nblk_total = sum(s[3] for s in schedule)
    in_maps = []
    for c in range(N_CORES):
        srcs = np.zeros((nblk_total, 128), np.int16)
        dstrel = np.zeros((128, nblk_total), np.float16)
        w0s = np.zeros((1, nblk_total), np.int32)
        bi = 0
        for (d, ch, rw, nb) in schedule:
            blks = core_blocks[c][d].get((ch, rw), [])
            for k in range(nb):
                s, r, w0 = blks[k] if k < len(blks) else _dummy_block()
                srcs[bi] = s
                dstrel[:, bi] = r
                w0s[0, bi] = w0 * D
                bi += 1
        assert bi == nblk_total

        # bias accumulator init [128, N_WIN*256] f32
        bias_vec = np.zeros((NODES_PAD, D), np.float32)
        lo, hi = c * NODES_PER_CORE, (c + 1) * NODES_PER_CORE
        cnt_f = np.zeros((NODES_PER_CORE, N_REL), np.float32)
        mf = (dep_idx >= lo) & (dep_idx < hi)
        np.add.at(cnt_f, (dep_idx[mf] - lo, rel_idx[mf]), 1.0)
        cnt_r = np.zeros((NODES_PER_CORE, N_REL), np.float32)
        mr = (gov_idx >= lo) & (gov_idx < hi)
        np.add.at(cnt_r, (gov_idx[mr] - lo, rel_idx[mr]), 1.0)
        bias_vec[:NODES_PER_CORE] = (
            np.asarray(b_self, np.float32)[None, :]
            + cnt_f @ np.asarray(b_fwd, np.float32)
            + cnt_r @ np.asarray(b_rev, np.float32))
        bias_mat = np.zeros((128, N_WIN * D), np.float32)
        for w in range(N_WIN):
            bias_mat[:, w * D:(w + 1) * D] = bias_vec[w * 128:(w + 1) * 128]

        meta = np.concatenate([iota, dstrel], axis=1)  # [128, 384+nblk]
        in_maps.append({
            "x16": x16,
            "wsb": wsb,
            "meta": meta,
            "w0s": w0s,
            "bias": bias_mat,
            "idx": _pack_idx16(srcs.reshape(-1)),
        })
    return schedule, nblk_total, in_maps


# ---------------------------------------------------------------- device

def build_bass(schedule, nblk_total):
    import os
    dbg = int(os.environ.get("GCN_DBG", "0"))  # 0=full 1=no-scatter 2=static-scatter
    nc = bacc.Bacc()
    x16 = nc.declare_dram_parameter("x16", [N_NODES, D], F16, isOutput=False)
    wsb = nc.declare_dram_parameter("wsb", [128, 21 * 2 * D], F16,
                                    isOutput=False)
    meta = nc.declare_dram_parameter("meta", [128, FRAME * WIN + nblk_total],
                                     F16, isOutput=False)
    w0s = nc.declare_dram_parameter("w0s", [1, nblk_total], I32,
                                    isOutput=False)
    bias = nc.declare_dram_parameter("bias", [128, N_WIN * D], F32,
                                     isOutput=False)
    idx = nc.declare_dram_parameter("idx", [128, nblk_total * 8], I16,
                                    isOutput=False)
    out = nc.declare_dram_parameter("out", [NODES_PER_CORE, D], F32,
                                    isOutput=True)

    # group schedule by (dir, chunk) for gather batching
    dc_groups = []  # (dir, chunk, [(relW, nb), ...], start_block)
    pos = 0
    for (d, ch, rw, nb) in schedule:
        if not dc_groups or dc_groups[-1][0] != d or dc_groups[-1][1] != ch:
            dc_groups.append([d, ch, [], pos])
        dc_groups[-1][2].append((rw, nb))
        pos += nb

    with TileContext(nc) as tc:
        with (
            tc.tile_pool(name="cst", bufs=1) as cst,
            tc.tile_pool(name="gp", bufs=2) as gp,
            tc.tile_pool(name="mp", bufs=3) as mp,
            tc.tile_pool(name="pp", bufs=1, space="PSUM") as pp,
            tc.tile_pool(name="pm", bufs=2, space="PSUM") as pm,
        ):
            wsb_t = cst.tile([128, 21 * 2 * D], F16, tag="wsb")
            nc.sync.dma_start(out=wsb_t[:], in_=wsb[:])
            meta_t = cst.tile([128, FRAME * WIN + nblk_total], F16, tag="meta")
            nc.sync.dma_start(out=meta_t[:], in_=meta[:])
            w0s_t = cst.tile([1, nblk_total], I32, tag="w0s")
            nc.sync.dma_start(out=w0s_t[:], in_=w0s[:])
            acc = cst.tile([128, N_WIN * D], F32, tag="acc")
            nc.sync.dma_start(out=acc[:], in_=bias[:])
            idx_t = cst.tile([128, nblk_total * 8], I16, tag="idx")
            nc.sync.dma_start(out=idx_t[:], in_=idx[:])
            zl = cst.tile([128, 128], F16, tag="zl")
            nc.vector.memset(zl[:], 0.0)
            zr = cst.tile([128, 512], F16, tag="zr")
            nc.vector.memset(zr[:], 0.0)

            iota_t = meta_t[:, 0:FRAME * WIN]

            psum_big = pp.tile([128, 12 * D], F32, tag="big")  # 6 banks

            max_nb = max(sum(nb for _, nb in g[2]) for g in dc_groups)

            GB = 7   # blocks/gather piece: 896 idxs -> s2m=114 <= 128 FIFO
            for (d, ch, rels, start) in dc_groups:
                nb_dc = sum(nb for _, nb in rels)
                nwin = CHUNK_WINS[ch]
                # transpose-gather source rows in FIFO-sized pieces
                g_pieces = []
                for p0 in range(0, nb_dc, GB):
                    pn = min(GB, nb_dc - p0)
                    g_p = gp.tile([128, 2, pn * 128], F16, tag="g")
                    if dbg == 4:
                        nc.vector.memset(g_p[:], 0.01)
                    else:
                        nc.gpsimd.dma_gather(
                            out_ap=g_p[:],
                            in_ap=x16[:],
                            idxs_ap=idx_t[:, (start + p0) * 8:
                                          (start + p0 + pn) * 8],
                            num_idxs=pn * 128,
                            num_idxs_reg=pn * 128,
                            elem_size=D,
                            transpose=True,
                        )
                    g_pieces.append(g_p)
                # init PSUM windows (has_written) for this chunk pass
                for k in range(6):
                    nc.tensor.matmul(out=psum_big[:, k * 512:(k + 1) * 512],
                                     lhsT=zl[:], rhs=zr[:],
                                     start=True, stop=False,
                                     skip_group_check=True)
                lb = 0
                for (rw, nb) in rels:
                    for _ in range(nb):
                        b = start + lb  # global block idx
                        # transform: M = G @ W_rw  [128e, 256o] fp32
                        g_p = g_pieces[lb // GB]
                        sub = lb % GB
                        m_ps = pm.tile([128, D], F32, tag="m")
                        nc.tensor.matmul(
                            out=m_ps[:],
                            lhsT=g_p[:, 0, sub * 128:(sub + 1) * 128],
                            rhs=wsb_t[:, (rw * 2) * D:(rw * 2 + 1) * D],
                            start=True, stop=False)
                        nc.tensor.matmul(
                            out=m_ps[:],
                            lhsT=g_p[:, 1, sub * 128:(sub + 1) * 128],
                            rhs=wsb_t[:, (rw * 2 + 1) * D:(rw * 2 + 2) * D],
                            start=False, stop=True)
                        m_sb = mp.tile([128, D], F16, tag="msb")
                        nc.scalar.copy(out=m_sb[:], in_=m_ps[:])
                        # selection matrix for the 3-window frame
                        s_t = mp.tile([128, FRAME * WIN], F16, tag="s")
                        col = FRAME * WIN + b
                        nc.vector.tensor_tensor(
                            out=s_t[:],
                            in0=meta_t[:, col:col + 1].to_broadcast(
                                [128, FRAME * WIN]),
                            in1=iota_t,
                            op=mybir.AluOpType.is_equal,
                        )
                        if dbg == 1:
                            lb += 1
                            continue
                        if dbg == 2:
                            for s in range(FRAME):
                                nc.tensor.matmul(
                                    out=psum_big[:, s * D:(s + 1) * D],
                                    lhsT=s_t[:, s * 128:(s + 1) * 128],
                                    rhs=m_sb[:],
                                    start=False, stop=False,
                                    skip_group_check=True,
                                )
                            lb += 1
                            continue
                        off = nc.values_load(
                            w0s_t[0:1, b:b + 1],
                            engines=[mybir.EngineType.PE],
                            min_val=0, max_val=(12 - FRAME) * D,
                            skip_runtime_bounds_check=True,
                        )
                        for s in range(FRAME):
                            nc.tensor.matmul(
                                out=psum_big[:, ds(off + s * D, D)],
                                lhsT=s_t[:, s * 128:(s + 1) * 128],
                                rhs=m_sb[:],
                                start=False, stop=False,
                                skip_group_check=True,
                            )
                        lb += 1
                # closing zero-matmuls: static-AP PE ops after all dynamic
                # scatter writes (PE FIFO order) so the flush reads below
                # carry a correct transitive dependency (PSUM-collision safe)
                for k in range(6):
                    nc.tensor.matmul(out=psum_big[:, k * 512:(k + 1) * 512],
                                     lhsT=zl[:], rhs=zr[:],
                                     start=False, stop=True,
                                     skip_group_check=True)
                # flush chunk windows into acc
                for wl in range(nwin):
                    wg = CHUNK_BASE_W[ch] + wl
                    nc.vector.tensor_add(
                        out=acc[:, wg * D:(wg + 1) * D],
                        in0=acc[:, wg * D:(wg + 1) * D],
                        in1=psum_big[:, wl * D:(wl + 1) * D],
                    )

            # write result: [3750, 256] f32
            n_full = NODES_PER_CORE // 128          # 29 full windows
            out3 = out[0:n_full * 128, :].rearrange("(w p) o -> p w o", p=128)
            acc3 = acc[:].rearrange("p (w o) -> p w o", o=D)[:, 0:n_full, :]
            nc.sync.dma_start(out=out3, in_=acc3)
            tail = NODES_PER_CORE - n_full * 128    # 38
            if tail:
                nc.sync.dma_start(
                    out=out[n_full * 128:NODES_PER_CORE, :],
                    in_=acc[0:tail, n_full * D:(n_full + 1) * D],
                )
    nc.finalize()
    return nc


# ---------------------------------------------------------------- entry

def kernel(x, W_self, b_self, W_fwd, b_fwd, W_rev, b_rev,
           dep_idx, rel_idx, gov_idx, _trace=False, _trace_kwargs=None):
    schedule, nblk_total, in_maps = prepare(
        x, W_self, b_self, W_fwd, b_fwd, W_rev, b_rev,
        dep_idx, rel_idx, gov_idx)
    nc = build_bass(schedule, nblk_total)
    res = run_bass_kernel_spmd(nc, in_maps, list(range(N_CORES)),
                               trace=_trace, **(_trace_kwargs or {}))
    out = np.concatenate([res.results[c]["out"] for c in range(N_CORES)],
                         axis=0)
    kernel._last_results = res
    return out.astype(np.float32)
